# revision 30
# baseline (speedup 1.0000x reference)
"""DynamicsNet Trainium2 kernel: 4 zero-state LSTM cells, data-parallel on 8 cores.

Reference math per row x[16]:
    h1 = relu(lstm1(x));  h2 = selu(lstm2(h1));  m = tanh(lstmM(h2));
    d = tanh(lstmD(h2));  out = concat([m, d], axis=0)
(zero-state LSTM cell: h = sigmoid(o) * tanh(sigmoid(i) * tanh(g)), f unused)

Layout: per core, 10 chunk-streams at 12-lane stride (partitions 12j..12j+11)
plus ONE shared constant lane at partition 120 that carries bias into every
matmul (stationary row 120) and is gap-engineered to survive the activation
pipeline (the ACT engine is the bottleneck, so rows-per-ACT-column is the
figure of merit: 10 chunks beats the 8x16 layout by 25%). cell1's 16 input
features are split across two accumulating matmul terms (xa: features 0-11 +
const lane; xb: features 12-15 on 40 partitions).

Stages stay fine-grained (3 PSUM ring banks each, ring of 8) — coarser
pairings measurably serialize the ring. Sigmoid goes through tanh:
sigma(z) = (1+tanh(z/2))/2 with the 1/2 folded into weights, so only the
Tanh+Exp ACT table set is used. selu is refactored to
(lambda/2 W)*(max(h2x,0) + min(exp(h2x/2)*2a, 2a) - 2a) with the -2a shift
folded into the m/d bias rows; exp is applied unclamped straight off the
h2x product to shorten the ACT<->DVE dependency chain.

The m/d FINAL tanh runs on the DVE as a degree-3 odd polynomial in fp16
fast modes (tanh(x/2) ~ x*(A + B*u), u=x^2) — the ACT engine is saturated,
the DVE had slack, and |hv| = 2|sig(o)tanh(c)| < 2 tanh(1) = 1.524 keeps
the fit error ~2e-3 on the narrow true domain.

Precision: single fp16 term everywhere (weights, x, h, gate tanh outputs)
+ fp16 outputs upcast on host; rel err 9.8e-3 vs the 2e-2 budget
(deterministic inputs, so the margin is stable).
"""

from contextlib import ExitStack

import numpy as np

LAMBDA = 1.0507009873554805
ALPHA = 1.6732632423543772

B, IN, H = 1048576, 16, 12
NCORES = 8
R = B // NCORES          # real rows per core (131072)
NCHUNK = 10              # chunk streams per core
F = 512                  # free-dim tile
NIT = 26                 # iterations
CLEN = NIT * F           # 13312 rows per stream (10*13312 = 133120 >= R)
LS = 12                  # lane stride per chunk
PCONST = 120             # shared constant lane
GAP_A = 2.0              # gap bias for I and O banks
V2 = 1.25                # engineered H2 const-lane value (fp16-exact)
TC_A = 0.4948864214517621    # tanh(x/2) ~ x*(A + B*u), u=x^2, |x|<1.53
TC_B = -0.03201637369728899   # (|hv|=2|sig(o)tanh(c)| < 2*tanh(1) = 1.524)

_CACHED = {}


def _solve_gap_g(target):
    """Gap bias for the G bank so the const lane's h2x equals `target`."""
    t_a = np.tanh(GAP_A)
    tc = target / (1.0 + t_a)
    c2 = 2.0 * np.arctanh(tc)
    tg = c2 / (1.0 + t_a)
    assert abs(tg) < 1.0
    return float(np.arctanh(tg))


def _prepare_consts(W_ih1, b_ih1, b_hh1, W_ih2, b_ih2, b_hh2,
                    W_ihm, b_ihm, b_hhm, W_ihd, b_ihd, b_hhd):
    i_s, g_s, o_s = slice(0, 12), slice(24, 36), slice(36, 48)
    g1gap = _solve_gap_g(1.0)    # H1 const lane -> 1.0
    g2gap = _solve_gap_g(V2)     # H2'' const lane -> V2 (positive branch)

    b1 = (b_ih1 + b_hh1).astype(np.float64)
    b2 = (b_ih2 + b_hh2).astype(np.float64)
    bm = (b_ihm + b_hhm).astype(np.float64)
    bd = (b_ihd + b_hhd).astype(np.float64)

    W1 = W_ih1.astype(np.float64)
    W2 = W_ih2.astype(np.float64)
    Wm = W_ihm.astype(np.float64)
    Wd = W_ihd.astype(np.float64)
    L2 = LAMBDA / 2.0

    # 15 stationary slots [128,128]: cell*3+bank for 4 cells, then 12+bank
    # for cell1's xb (features 12-15) term.
    w_np = np.zeros((128, 15 * 128), np.float16)

    def put(slot, m):
        w_np[:, 128 * slot:128 * slot + 128] = m.astype(np.float16)

    for bank, (gsl, sc) in enumerate(((i_s, 0.5), (g_s, 1.0), (o_s, 0.5))):
        gv1 = {0: GAP_A, 1: g1gap, 2: GAP_A}[bank]
        gv2 = {0: GAP_A, 1: g2gap, 2: GAP_A}[bank]

        # cell1 A-term: features 0-11, bias row, const-lane seed
        ma = np.zeros((128, 128), np.float64)
        mb = np.zeros((128, 128), np.float64)
        for j in range(NCHUNK):
            c = slice(LS * j, LS * j + 12)
            ma[LS * j:LS * j + 12, c] = (W1[gsl, 0:12] * sc).T
            mb[4 * j:4 * j + 4, c] = (W1[gsl, 12:16] * sc).T
            ma[PCONST, c] = b1[gsl] * sc
        ma[PCONST, PCONST] = gv1
        put(0 * 3 + bank, ma)
        put(12 + bank, mb)

        # cell2: input H1 = 2*relu(h1) -> extra 1/2; H1 const lane = 1.0
        m2 = np.zeros((128, 128), np.float64)
        for j in range(NCHUNK):
            c = slice(LS * j, LS * j + 12)
            m2[LS * j:LS * j + 12, c] = (W2[gsl] * (sc * 0.5)).T
            m2[PCONST, c] = b2[gsl] * sc
        m2[PCONST, PCONST] = gv2
        put(1 * 3 + bank, m2)

        # m/d: input H2'' -> scale lambda/2; H2'' const lane = V2
        for cell, (W, bb) in ((2, (Wm, bm)), (3, (Wd, bd))):
            mm = np.zeros((128, 128), np.float64)
            for j in range(NCHUNK):
                c = slice(LS * j, LS * j + 12)
                mm[LS * j:LS * j + 12, c] = (W[gsl] * (sc * L2)).T
                mm[PCONST, c] = bb[gsl] * sc / V2
            put(cell * 3 + bank, mm)
    return w_np


def _build_bass():
    import concourse.bass as bass
    import concourse.mybir as mybir
    import concourse.tile as tile

    fp32 = mybir.dt.float32
    fp16 = mybir.dt.float16
    Tanh = mybir.ActivationFunctionType.Tanh
    Exp = mybir.ActivationFunctionType.Exp
    ADD = mybir.AluOpType.add
    MULT = mybir.AluOpType.mult
    MIN = mybir.AluOpType.min
    MAX = mybir.AluOpType.max
    TWOA = float(2.0 * ALPHA)

    nc = bass.Bass(name="dynet")
    xa_dev = nc.dram_tensor("xa_dev", [128, CLEN], fp16, kind="ExternalInput")
    xb_dev = nc.dram_tensor("xb_dev", [40, CLEN], fp16, kind="ExternalInput")
    w_dram = nc.dram_tensor("w_dram", [128, 15 * 128], fp16, kind="ExternalInput")
    md_dev = nc.dram_tensor("md_dev", [128, 2 * CLEN], fp16,
                            kind="ExternalOutput")

    with tile.TileContext(nc) as tc, ExitStack() as ctx:
        const_p = ctx.enter_context(tc.tile_pool(name="const", bufs=1))
        xp = ctx.enter_context(tc.tile_pool(name="x", bufs=4))
        Tp = ctx.enter_context(tc.tile_pool(name="T", bufs=3))
        smallp = ctx.enter_context(tc.tile_pool(name="small", bufs=5))
        hp = ctx.enter_context(tc.tile_pool(name="h", bufs=5))
        psp = ctx.enter_context(tc.tile_pool(name="ps", bufs=1, space="PSUM"))

        wsb = const_p.tile([128, 15 * 128], fp16)
        nc.sync.dma_start(wsb[:], w_dram[:])
        ebt = const_p.tile([128, 1], fp32)
        nc.vector.memset(ebt[:], float(np.log(2.0 * ALPHA)))

        # PSUM ring of 8 banks; each stage claims 3 consecutive (mod 8).
        pp_all = psp.tile([128, 4096], fp32)

        def bank(s, b):
            w = ((3 * s + b) % 8) * 512
            return pp_all[:, w:w + 512]

        def mms(s, cell, rhs):
            """3 full-array MMs: one fp16 term per gate bank."""
            for b in range(3):
                w = cell * 3 + b
                nc.tensor.matmul(
                    bank(s, b), wsb[:, 128 * w:128 * w + 128], rhs[:, :],
                    start=True, stop=True)

        def mms_c1(s, rhs_a, rhs_b):
            """cell1: 2 accumulating terms per bank (features 0-11 via xa
            incl bias/const rows, features 12-15 via xb on 40 partitions)."""
            for b in range(3):
                nc.tensor.matmul(
                    bank(s, b), wsb[:, 128 * b:128 * b + 128], rhs_a[:, :],
                    start=True, stop=False)
                nc.tensor.matmul(
                    bank(s, b), wsb[0:40, 128 * (12 + b):128 * (12 + b) + 128],
                    rhs_b[:, :], start=False, stop=True)

        def gate_act(s, T, n=3):
            """tanh over the stage's n ring banks -> T[:, 0:512*n]."""
            base = (3 * s) % 8
            n1 = min(n, 8 - base)
            nc.scalar.activation(
                T[:, 0:512 * n1],
                pp_all[:, 512 * base:512 * (base + n1)], Tanh)
            if n1 < n:
                nc.scalar.activation(
                    T[:, 512 * n1:512 * n], pp_all[:, 0:512 * (n - n1)], Tanh)

        xa = {}
        xb = {}
        H1 = {}
        H2 = {}
        TMD = {}
        sctr = 0
        # fin(k): [0:1024]=hmd(k-4), [1024:2048]=c2md(k-3),
        #         [2048:2560]=c2_2(k-1), [2560:3072]=c2_1(k)
        fprev = None
        for k in range(NIT + 4):
            if k < NIT:
                xa[k] = xp.tile([128, F], fp16, tag="xa", name=f"xa{k}")
                nc.sync.dma_start(xa[k][:], xa_dev[:, F * k:F * (k + 1)])
                xb[k] = xp.tile([40, F], fp16, tag="xb", name=f"xb{k}")
                nc.sync.dma_start(xb[k][:], xb_dev[:, F * k:F * (k + 1)])

            fin = smallp.tile([128, 3072], fp16, tag="fin", bufs=5, name=f"fin{k}")
            fout = smallp.tile([128, 3072], fp16, tag="fout", bufs=5, name=f"fout{k}")

            # --- m/d matmuls + gates + c2 for iteration k-3 ---
            if 3 <= k < NIT + 3:
                it = k - 3
                sm, sd = sctr, sctr + 1
                sctr += 2
                mms(sm, 2, H2[it])
                mms(sd, 3, H2[it])
                Tmd = Tp.tile([128, 3072], fp16, tag="Tmd", bufs=5,
                              name=f"Tmd{it}")
                gate_act(sm, Tmd[:, 0:1536])
                gate_act(sd, Tmd[:, 1536:3072])
                nc.vector.scalar_tensor_tensor(
                    fin[:, 1024:1536], Tmd[:, 0:512], 1.0, Tmd[:, 512:1024],
                    op0=ADD, op1=MULT)
                nc.vector.scalar_tensor_tensor(
                    fin[:, 1536:2048], Tmd[:, 1536:2048], 1.0,
                    Tmd[:, 2048:2560], op0=ADD, op1=MULT)
                TMD[it] = Tmd
                del H2[it]

            # --- hmd for iteration k-4 (needs tcmd from fout(k-1)),
            # then final tanh via deg-5 odd poly on DVE (ACT is the pacer) ---
            if 4 <= k:
                it2 = k - 4
                Tmd_p = TMD.pop(it2)
                hv = smallp.tile([128, 1024], fp16, tag="hv", name=f"hv{k}")
                nc.vector.scalar_tensor_tensor(
                    hv[:, 0:512], Tmd_p[:, 1024:1536], 1.0,
                    fprev[:, 1024:1536], op0=ADD, op1=MULT)
                nc.vector.scalar_tensor_tensor(
                    hv[:, 512:1024], Tmd_p[:, 2560:3072], 1.0,
                    fprev[:, 1536:2048], op0=ADD, op1=MULT)
                ucb = smallp.tile([128, 1024], fp16, tag="ucb")
                qcb = smallp.tile([128, 1024], fp16, tag="qcb")
                mdo = smallp.tile([128, 1024], fp16, tag="mdo",
                                  name=f"mdo{k}")
                nc.vector.tensor_tensor(ucb[:, :], hv[:, :], hv[:, :],
                                        op=MULT)
                nc.vector.tensor_scalar(qcb[:, :], ucb[:, :], TC_B, TC_A,
                                        op0=MULT, op1=ADD)
                nc.vector.tensor_tensor(mdo[:, :], qcb[:, :], hv[:, :],
                                        op=MULT)

            # --- cell2 matmul + gates + c2 for iteration k-1 ---
            if 1 <= k <= NIT:
                it = k - 1
                s2 = sctr
                sctr += 1
                mms(s2, 1, H1[it])
                T2 = Tp.tile([128, 1536], fp16, tag="T", bufs=5)
                gate_act(s2, T2)
                nc.vector.scalar_tensor_tensor(
                    fin[:, 2048:2560], T2[:, 0:512], 1.0, T2[:, 512:1024],
                    op0=ADD, op1=MULT)

            # --- cell1 matmuls + gates + c2 for iteration k ---
            if k < NIT:
                s1 = sctr
                sctr += 1
                mms_c1(s1, xa[k], xb[k])
                T1 = Tp.tile([128, 1536], fp16, tag="T", bufs=5)
                gate_act(s1, T1)
                nc.vector.scalar_tensor_tensor(
                    fin[:, 2560:3072], T1[:, 0:512], 1.0, T1[:, 512:1024],
                    op0=ADD, op1=MULT)

            # --- merged tanh(0.5*x): c2md piece + c2both piece ---
            if 3 <= k < NIT + 3:
                nc.scalar.activation(fout[:, 1024:2048], fin[:, 1024:2048],
                                     Tanh, scale=0.5)
            lo2 = 2048 if 1 <= k else 2560
            hi2 = 3072 if k < NIT else (2560 if k == NIT else 2048)
            if lo2 < hi2:
                nc.scalar.activation(fout[:, lo2:hi2], fin[:, lo2:hi2], Tanh,
                                     scale=0.5)
            if 4 <= k:
                it2 = k - 4
                nc.sync.dma_start(md_dev[:, 1024 * it2:1024 * (it2 + 1)],
                                  mdo[:, :])

            # --- cell2 epilogue: h2x -> H2'' (selu refactor) ---
            if 1 <= k <= NIT:
                it = k - 1
                h2x2 = smallp.tile([128, F], fp16, tag="h2x2",
                                   name=f"h2x2_{k}")
                nc.vector.scalar_tensor_tensor(
                    h2x2[:, :], T2[:, 1024:1536], 1.0, fout[:, 2048:2560],
                    op0=ADD, op1=MULT)
                e2t = smallp.tile([128, F], fp32, tag="e2")
                rlu = smallp.tile([128, F], fp32, tag="rlu")
                h2h = hp.tile([128, F], fp16, tag="H2h", name=f"H2h_{it}")
                nc.scalar.activation(e2t[:, :], h2x2[:, :], Exp,
                                     bias=ebt[:, :], scale=0.5)
                nc.vector.tensor_scalar(rlu[:, :], h2x2[:, :], 0.0, -TWOA,
                                        op0=MAX, op1=ADD)
                nc.vector.scalar_tensor_tensor(
                    h2h[:, :], e2t[:, :], TWOA, rlu[:, :],
                    op0=MIN, op1=ADD)
                H2[it] = h2h
                del H1[it]

            # --- cell1 epilogue: h2x -> H1 ---
            if k < NIT:
                h2x1 = smallp.tile([128, F], fp16, tag="h2x1",
                                   name=f"h2x1_{k}")
                nc.vector.scalar_tensor_tensor(
                    h2x1[:, :], T1[:, 1024:1536], 1.0, fout[:, 2560:3072],
                    op0=ADD, op1=MULT)
                h1h = hp.tile([128, F], fp16, tag="H1h", name=f"H1h_{k}")
                nc.vector.tensor_scalar_max(h1h[:, :], h2x1[:, :], 0.0)
                H1[k] = h1h
                del xa[k], xb[k]

            fprev = fout

    _legalize_waits(nc)
    return nc


def _legalize_waits(nc):
    """Split multi-wait instructions into single-wait same-engine NoOps
    (the cayman ISA has one sync-wait slot per instruction)."""
    import concourse.mybir as mybir
    n = 0
    for func in nc.m.functions:
        for blk in func.blocks:
            out = []
            changed = False
            for inst in blk.instructions:
                si = inst.sync_info
                waits = list(si.on_wait) if si is not None and si.on_wait else []
                if len(waits) > 1:
                    changed = True
                    for w in waits[:-1]:
                        n += 1
                        nop = mybir.InstNoOp(name=f"legw-{n}", ins=[], outs=[])
                        nop.engine = inst.engine
                        nop.sync_info = mybir.SyncInfo(on_wait=[w], on_update=[])
                        out.append(nop)
                    inst.sync_info = mybir.SyncInfo(
                        on_wait=[waits[-1]],
                        on_update=list(si.on_update) if si.on_update else [])
                out.append(inst)
            if changed:
                blk.instructions = out
    return n


def _run(x, consts, trace=False):
    from concourse.bass_utils import run_bass_kernel_spmd

    if "nc" not in _CACHED:
        _CACHED["nc"] = _build_bass()
    nc = _CACHED["nc"]
    w_np = consts

    in_maps = []
    for c in range(NCORES):
        xpad = np.zeros((NCHUNK * CLEN, IN), np.float32)
        xpad[:R] = x[c * R:(c + 1) * R]
        arr = np.ascontiguousarray(
            xpad.reshape(NCHUNK, CLEN, IN).transpose(0, 2, 1))  # [C,16,CLEN]
        xa = np.zeros((128, CLEN), np.float16)
        xb = np.zeros((40, CLEN), np.float16)
        for j in range(NCHUNK):
            xa[LS * j:LS * j + 12] = arr[j, 0:12]
            xb[4 * j:4 * j + 4] = arr[j, 12:16]
        xa[PCONST] = 1.0
        in_maps.append({"xa_dev": xa, "xb_dev": xb, "w_dram": w_np})

    res = run_bass_kernel_spmd(nc, in_maps, core_ids=list(range(NCORES)),
                               trace=trace)

    out = np.empty((2 * B, H), np.float32)
    lanes = np.concatenate([np.arange(LS * j, LS * j + 12)
                            for j in range(NCHUNK)])
    for c in range(NCORES):
        md = res.results[c]["md_dev"].astype(np.float32)  # [128, 2*CLEN]
        md = md.reshape(128, NIT, 2, F)
        for half, base in ((0, 0), (1, B)):
            dev = md[:, :, half, :].reshape(128, CLEN)
            full = dev[lanes].reshape(NCHUNK, 12, CLEN)
            rows = full.transpose(0, 2, 1).reshape(NCHUNK * CLEN, H)
            out[base + c * R: base + (c + 1) * R] = rows[:R]
    return out, res


def kernel(x, W_ih1, b_ih1, b_hh1, W_ih2, b_ih2, b_hh2,
           W_ihm, b_ihm, b_hhm, W_ihd, b_ihd, b_hhd):
    x = np.asarray(x, np.float32)
    consts = _prepare_consts(
        np.asarray(W_ih1, np.float32), np.asarray(b_ih1, np.float32),
        np.asarray(b_hh1, np.float32), np.asarray(W_ih2, np.float32),
        np.asarray(b_ih2, np.float32), np.asarray(b_hh2, np.float32),
        np.asarray(W_ihm, np.float32), np.asarray(b_ihm, np.float32),
        np.asarray(b_hhm, np.float32), np.asarray(W_ihd, np.float32),
        np.asarray(b_ihd, np.float32), np.asarray(b_hhd, np.float32))
    out, _ = _run(x, consts, trace=False)
    return out
